# revision 15
# baseline (speedup 1.0000x reference)
"""RNN-T joint network kernel for Trainium2 (8 NeuronCores).

Math (B,T,U,H,V = 4,300,64,512,1024):
  hx = x @ W1[:512];  hy = y @ W1[512:]
  gx = x @ Wg[:512];  gy = y @ Wg[512:]
  z  = tanh(hx[:,:,None,:] + hy[:,None,:,:] + b1)
  g  = sigmoid(gx[...] + gy[...] + bg)        # = 0.5*(1+tanh(mid/2))
  P  = (z*g) @ W2 + b2
  out = log_softmax(P, axis=-1)

Device strategy:
  - The small projections (hx,hy,gx,gy) are computed host-side (numpy); the
    device does the O(B*T*U*(H+V)) work: broadcast-add via selector matmuls,
    tanh, z*g fusion, the big (.,512)@(512,1024) matmul, and log-softmax.
  - sigmoid folded as tanh: gx,gy pre-scaled by 0.5 host-side, and 0.5
    folded into W2 (z*g = z*0.5*(1+th_g) -> m'' = th_z*(1+th_g), W2h=0.5*W2).
  - Transposed orientation: activations live as [h mod 128, (h-chunk, row)]
    so m'' is directly a matmul lhsT (no transposes anywhere).
  - log-softmax: exp with accum_out gives S; ln(S) via bitcast-exponent
    initial guess + 2 Newton iterations (y <- y + S*exp(-y) - 1).
  - Sharding: core c -> batch b=c//2, T-half half=c%2 (150 t-values each).
"""

import os
import sys

import numpy as np

sys.path.insert(0, "/opt/trn_rl_repo")
os.environ.setdefault("MYCRO_LOCAL_CACHE", "1")

B, T, U, H, V = 4, 300, 64, 512, 1024
TC = T // 2          # t-values per core (150)
ROWS = TC * U        # output rows per core (9600)
LN2 = 0.6931471805599453
C1 = LN2 / (1 << 23)
C2 = 0.02983 - 127.0 * LN2

# (block, t0-within-block, nt): 16 macros of 8 t's over block 0 (t 0..127),
# then 8+8+6 over block 1 (t 128..149).
MACROS = [(0, 8 * m, 8) for m in range(16)] + [(1, 0, 8), (1, 8, 8), (1, 16, 6)]

_CACHE = {}


def _build(with_b2: bool):
    if with_b2 in _CACHE:
        return _CACHE[with_b2]

    from contextlib import ExitStack

    from concourse import bacc, mybir
    import concourse.tile as tile

    dt = mybir.dt
    f32 = dt.float32
    f32r = dt.float32r
    i32 = dt.int32
    AF = mybir.ActivationFunctionType
    OP = mybir.AluOpType

    nc = bacc.Bacc(
        "TRN2",
        target_bir_lowering=False,
        debug=False,
        enable_asserts=True,
        num_devices=8,
    )

    n_mac = len(MACROS)
    hx_d = nc.dram_tensor("hx", (8, n_mac, H), f32r, kind="ExternalInput").ap()
    gx_d = nc.dram_tensor("gx", (8, n_mac, H), f32r, kind="ExternalInput").ap()
    hy_d = nc.dram_tensor("hy", (U, H), f32r, kind="ExternalInput").ap()
    gy_d = nc.dram_tensor("gy", (U, H), f32r, kind="ExternalInput").ap()
    w2_d = nc.dram_tensor("w2", (128, 4, V), f32r, kind="ExternalInput").ap()
    e8_d = nc.dram_tensor("e8", (8, 512), f32r, kind="ExternalInput").ap()
    eu_d = nc.dram_tensor("eu", (U, 512), f32r, kind="ExternalInput").ap()
    if with_b2:
        b2_d = nc.dram_tensor("b2r", (1, V), f32r, kind="ExternalInput").ap()
    out_d = nc.dram_tensor("out", (ROWS, 2, 512), f32, kind="ExternalOutput").ap()

    with tile.TileContext(nc) as tc, ExitStack() as ctx:
        consts = ctx.enter_context(tc.tile_pool(name="consts", bufs=1))
        work = ctx.enter_context(tc.tile_pool(name="work", bufs=2))
        small = ctx.enter_context(tc.tile_pool(name="small", bufs=4))
        outp = ctx.enter_context(tc.tile_pool(name="outp", bufs=3))
        ppre = ctx.enter_context(tc.tile_pool(name="ppre", bufs=2, space="PSUM"))
        ppp = ctx.enter_context(tc.tile_pool(name="ppp", bufs=2, space="PSUM"))

        hx_t = consts.tile((8, n_mac, H), f32r, tag="hx")
        gx_t = consts.tile((8, n_mac, H), f32r, tag="gx")
        hy_t = consts.tile((U, H), f32r, tag="hy")
        gy_t = consts.tile((U, H), f32r, tag="gy")
        w2_t = consts.tile((128, 4, V), f32r, tag="w2")
        e8_t = consts.tile((8, 512), f32r, tag="e8")
        eu_t = consts.tile((U, 512), f32r, tag="eu")

        nc.sync.dma_start(w2_t[:], w2_d[:])
        nc.gpsimd.dma_start(hx_t[:], hx_d[:])
        nc.gpsimd.dma_start(gx_t[:], gx_d[:])
        nc.sync.dma_start(hy_t[:], hy_d[:])
        nc.sync.dma_start(gy_t[:], gy_d[:])
        nc.gpsimd.dma_start(e8_t[:], e8_d[:])
        nc.gpsimd.dma_start(eu_t[:], eu_d[:])
        if with_b2:
            b2_t = consts.tile((1, V), f32r, tag="b2r")
            ones_t = consts.tile((1, 128), f32r, tag="ones")
            nc.sync.dma_start(b2_t[:], b2_d[:])
            nc.vector.memset(ones_t[:], 1.0)

        dma_i = 0
        for mi, (blk, t0, nt) in enumerate(MACROS):
            nr = nt * U
            nsub = nr // 128
            t_base = blk * 128 + t0

            th = {}
            for name, xa, ya in (("z", hx_t, hy_t), ("g", gx_t, gy_t)):
                tht = work.tile((128, 4, 512), f32, tag="th_" + name)
                for half in (0, 1):
                    pre = ppre.tile((128, 2, 512), f32, tag="pre")
                    for ci in (0, 1):
                        c = 2 * half + ci
                        nc.tensor.matmul(
                            pre[:, ci, 0:nr],
                            xa[0:nt, mi, c * 128 : (c + 1) * 128],
                            e8_t[0:nt, 0:nr],
                            start=True,
                            stop=False,
                        )
                        nc.tensor.matmul(
                            pre[:, ci, 0:nr],
                            ya[0:U, c * 128 : (c + 1) * 128],
                            eu_t[0:U, 0:nr],
                            start=False,
                            stop=True,
                        )
                    nc.scalar.activation(
                        tht[:, 2 * half : 2 * half + 2, 0:nr],
                        pre[:, :, 0:nr],
                        AF.Tanh,
                    )
                th[name] = tht

            m2 = work.tile((128, 4, 512), f32r, tag="m2")
            for j in range(nsub):
                js = slice(j * 128, (j + 1) * 128)
                # m'' = th_z * (1 + th_g)
                nc.vector.scalar_tensor_tensor(
                    m2[:, :, js], th["g"][:, :, js], 1.0, th["z"][:, :, js],
                    OP.add, OP.mult,
                )
                pp = ppp.tile((128, 2, 512), f32, tag="pp")
                for vh in (0, 1):
                    for c in range(4):
                        nc.tensor.matmul(
                            pp[:, vh, :],
                            m2[:, c, js],
                            w2_t[:, c, vh * 512 : (vh + 1) * 512],
                            start=(c == 0),
                            stop=(c == 3 and not with_b2),
                        )
                    if with_b2:
                        nc.tensor.matmul(
                            pp[:, vh, :],
                            ones_t[:],
                            b2_t[0:1, vh * 512 : (vh + 1) * 512],
                            start=False,
                            stop=True,
                        )
                # softmax denominator
                scr = work.tile((128, 2, 512), f32, tag="scr")
                s = small.tile((128, 1), f32, tag="s")
                nc.scalar.activation(scr[:], pp[:], AF.Exp, accum_out=s[:])
                # ln(S): exponent-bits initial guess + 2 Newton steps
                ib = small.tile((128, 1), f32, tag="ib")
                nc.vector.tensor_copy(ib[:], s[:].bitcast(i32))
                yc = small.tile((128, 1), f32, tag="y0")
                nc.vector.tensor_scalar(yc[:], ib[:], C1, C2, OP.mult, OP.add)
                for it in range(2):
                    e = small.tile((128, 1), f32, tag="nw_e")
                    nc.scalar.activation(e[:], yc[:], AF.Exp, scale=-1.0)
                    u = small.tile((128, 1), f32, tag="nw_u")
                    nc.vector.tensor_tensor(u[:], s[:], e[:], OP.mult)
                    yn = small.tile((128, 1), f32, tag="nw_y%d" % it)
                    nc.vector.scalar_tensor_tensor(
                        yn[:], u[:], -1.0, yc[:], OP.add, OP.add
                    )
                    yc = yn
                ob = outp.tile((128, 2, 512), f32, tag="ob")
                nc.vector.tensor_scalar(ob[:], pp[:], yc[:], None, OP.subtract)
                r0 = t_base * U + j * 128
                eng = nc.sync if dma_i % 2 == 0 else nc.gpsimd
                eng.dma_start(out_d[r0 : r0 + 128, :, :], ob[:])
                dma_i += 1

    nc.compile()
    _CACHE[with_b2] = nc
    return nc


_LAST = None


def _host_prep(inputs):
    f32 = np.float32
    x = inputs["x"].astype(f32, copy=False)
    y = inputs["y"].astype(f32, copy=False)
    W1 = inputs["W1"].astype(f32, copy=False)
    Wg = inputs["Wg"].astype(f32, copy=False)
    W2 = inputs["W2"].astype(f32, copy=False)
    b1 = inputs["b1"].astype(f32, copy=False)
    bg = inputs["bg"].astype(f32, copy=False)
    b2 = inputs["b2"].astype(f32, copy=False)

    # host-side projections (cheap relative to device work)
    hx = (x.reshape(B * T, H) @ W1[:H] + b1).reshape(B, T, H)
    gx = ((x.reshape(B * T, H) @ Wg[:H]) * 0.5).reshape(B, T, H)
    hy = y.reshape(B * U, H) @ W1[H:]
    gy = ((y.reshape(B * U, H) @ Wg[H:] + bg) * 0.5)
    hy = hy.reshape(B, U, H)
    gy = gy.reshape(B, U, H)

    w23 = np.ascontiguousarray((0.5 * W2).reshape(4, 128, V).transpose(1, 0, 2))
    e8 = np.zeros((8, 512), f32)
    for t in range(8):
        e8[t, t * U : (t + 1) * U] = 1.0
    eu = np.ascontiguousarray(np.tile(np.eye(U, dtype=f32), (1, 8)))

    with_b2 = bool(np.any(b2))

    in_maps = []
    for c in range(8):
        b, half = divmod(c, 2)
        n_mac = len(MACROS)
        hxc = np.zeros((8 * n_mac, H), f32)
        gxc = np.zeros((8 * n_mac, H), f32)
        hxc[0:TC] = hx[b, half * TC : (half + 1) * TC]
        gxc[0:TC] = gx[b, half * TC : (half + 1) * TC]
        # MACROS cover consecutive t runs of size 8,8,...,8,6 -> macro mi
        # starts at t = 8*mi (last macro uses only 6 of its 8 rows)
        hx2 = np.ascontiguousarray(hxc.reshape(n_mac, 8, H).transpose(1, 0, 2))
        gx2 = np.ascontiguousarray(gxc.reshape(n_mac, 8, H).transpose(1, 0, 2))
        m = {
            "hx": hx2,
            "gx": gx2,
            "hy": np.ascontiguousarray(hy[b]),
            "gy": np.ascontiguousarray(gy[b]),
            "w2": w23,
            "e8": e8,
            "eu": eu,
        }
        if with_b2:
            m["b2r"] = np.ascontiguousarray(b2.reshape(1, V))
        in_maps.append(m)
    return in_maps, with_b2


def kernel(**inputs: np.ndarray) -> np.ndarray:
    global _LAST
    f32 = np.float32
    in_maps, with_b2 = _host_prep(inputs)
    nc = _build(with_b2)
    from concourse.bass_utils import run_bass_kernel_spmd

    trace = os.environ.get("RNNT_TRACE") == "1"
    try:
        res = run_bass_kernel_spmd(nc, in_maps, core_ids=list(range(8)), trace=trace)
    except ModuleNotFoundError:
        # NTFF profile hook unavailable in this container; run without trace
        res = run_bass_kernel_spmd(nc, in_maps, core_ids=list(range(8)), trace=False)
    _LAST = res

    outf = np.empty((B, T, U, V), f32)
    for c in range(8):
        b, half = divmod(c, 2)
        outf[b, half * TC : (half + 1) * TC] = res.results[c]["out"].reshape(TC, U, V)
    return outf
